# revision 16
# baseline (speedup 1.0000x reference)
"""Trainium2 Bass kernel for the DCRF mean-field iteration module.

Math: the (B,N,N) pairwise potential is separable:
    PP[b,i,j] = g_i * g_j * (1 - u_i.u_j) * Wsym[i,j]
with g = exp(-|f|^2/2), u = f/|f| (2-component), Wsym = (W + W^T)/2.
Each mean-field step reduces sum_j PP[i,j] * v_j (v = tanh(logits/2)) to
    E_i = g_i*(S0_i) - g_i*ux_i*Sx_i - g_i*uy_i*Sy_i,
    [S0 Sx Sy] = Wsym @ [g*v, g*ux*v, g*uy*v]
i.e. one (N,N)@(N,3B) matmul per iteration instead of a 512MB tensor.

Sharding: W's columns are sharded 8 ways; core k owns output rows
[512k, 512k+512).  The slab is staged as fp8e4m3 at scale 64 (host-side
(W+W^T)*64 cast; the 1/64 is folded into the combine coefficients), which
numpy-simulates to rel err ~4e-3 vs the 2e-2 gate.  PE matmuls run in
DoubleRow mode (2 fp8 weights per cell, K virtualized to 256): 16 MMs of
[128,2,32] x [128,2,512] per iteration, X stationary, slab moving.

Per iteration: only v (tanh of logits, [512,8] bf16 = 8KB) is AllGathered;
every core keeps a replicated gst table (g, g*ux, g*uy, 0 for all 4096
nodes, bf16) and rebuilds the full fp8 X with one broadcast-multiply.
Stationary columns are b-major (m = 4b + s) so the combine is a single
PSUM-read multiply + innermost-dim reduce.  Iteration 1 needs no
collective (v0 = tanh(unary/2) from the replicated full logits).
Optional dependency-anchored junk-MM groups (KWARM_A/KWARM_B, default 0)
can pace PE activity through the AllGather gap; measured net-negative
here (223us without vs 238us with), so they are off.
"""

import os
import sys

import numpy as np

for _p in ("/opt/trn_rl_repo", "/root/.axon_site/_ro/trn_rl_repo"):
    if os.path.isdir(_p) and _p not in sys.path:
        sys.path.insert(0, _p)

import concourse.bass as bass  # noqa: E402
import concourse.tile as tile  # noqa: E402
from concourse import bacc  # noqa: E402
from concourse import mybir  # noqa: E402
from concourse.bass_utils import run_bass_kernel_spmd  # noqa: E402
from concourse.masks import make_identity  # noqa: E402

B = 8          # batch
G = 64         # grid
N = G * G      # 4096 nodes
ITER = 10
NCORES = 8
R = N // NCORES        # 512 own rows per core
DCH = R // 128         # 4 own 128-row blocks
NCH = N // 128         # 32 contraction chunks
QCH = NCH // 2         # 16 DoubleRow chunk pairs
S4 = 4                 # stats per node: a, bx, by, 0-pad
MC = S4 * B            # 32 stationary columns, m = 4*b + s
WARM_A = int(os.environ.get("KWARM_A", "0"))    # junk MMs after transposes
WARM_B = int(os.environ.get("KWARM_B", "0"))    # junk MMs gated on DMA-out
WSCALE = 64.0          # fp8 staging scale for the W slab

F32 = mybir.dt.float32
BF16 = mybir.dt.bfloat16
FP8 = mybir.dt.float8e4


def _declare_io(nc):
    # Per-core inputs (host-sharded views of the full inputs).  The W slab
    # arrives pre-symmetrized, scaled and cast to fp8e4m3, in DoubleRow
    # layout [p, pair, ko, own_i] with j = p*32 + 2*pair + ko.
    w_dr = nc.dram_tensor("w_dr", [128, QCH, 2, R], FP8, kind="ExternalInput")
    dp_own = nc.dram_tensor("dp_own", [2, R, B], F32, kind="ExternalInput")
    dp_full = nc.dram_tensor("dp_full", [2, N, B], F32, kind="ExternalInput")
    lg_own = nc.dram_tensor("lg_own", [R, B], F32, kind="ExternalInput")
    lg_full = nc.dram_tensor("lg_full", [N, B], F32, kind="ExternalInput")
    out_own = nc.dram_tensor("out_own", [R, B], F32, kind="ExternalOutput")
    return w_dr, dp_own, dp_full, lg_own, lg_full, out_own


def _make_in_maps(delta_p, logits, W):
    import ml_dtypes
    delta_p = np.ascontiguousarray(np.asarray(delta_p, dtype=np.float32))
    logits = np.ascontiguousarray(np.asarray(logits, dtype=np.float32))
    W = np.ascontiguousarray(np.asarray(W, dtype=np.float32))
    feats = delta_p.reshape(B, N, 2)
    dp_full = np.ascontiguousarray(feats.transpose(2, 1, 0))  # [comp, node, b]
    lg_full = np.ascontiguousarray(logits[:, :, 0].T)         # [node, batch]
    w2 = (W[0] + W[0].T) * WSCALE                             # 2*Wsym*scale
    in_maps = []
    for k in range(NCORES):
        rows = slice(R * k, R * (k + 1))
        wdr = w2[:, rows].reshape(128, QCH, 2, R)
        in_maps.append({
            "w_dr": np.ascontiguousarray(wdr).astype(ml_dtypes.float8_e4m3),
            # (2, R, B): [component, own row, batch]
            "dp_own": np.ascontiguousarray(feats[:, rows, :].transpose(2, 1, 0)),
            "dp_full": dp_full,
            "lg_own": np.ascontiguousarray(lg_full[rows]),
            "lg_full": lg_full,
        })
    return in_maps


def _assemble_out(res):
    out = np.empty((B, N, 1), dtype=np.float32)
    for k, r in enumerate(res.results):
        out[:, R * k:R * (k + 1), 0] = r["out_own"].T
    return out


def _build_kernel():
    nc = bacc.Bacc("TRN2", target_bir_lowering=False, debug=False,
                   num_devices=NCORES)
    tensors = _declare_io(nc)
    with tile.TileContext(nc) as tc:
        _emit(tc, nc, *[t.ap() for t in tensors])
    nc.compile()
    return nc


def _emit(tc, nc, w_dr, dp_own, dp_full, lg_own, lg_full, out_own,
          chain_after=None, comm=True):
    import contextlib
    from concourse.tile_rust import add_dep_helper

    entry = []  # input-loading instructions (for benchmark serialization)

    ctx = contextlib.ExitStack()
    with ctx:
        singles = ctx.enter_context(tc.tile_pool(name="singles", bufs=1))
        small = ctx.enter_context(tc.tile_pool(name="small", bufs=3))
        vpool = ctx.enter_context(tc.tile_pool(name="vpool", bufs=2))
        psum = ctx.enter_context(tc.tile_pool(name="psum", bufs=2, space="PSUM"))
        psum_t = ctx.enter_context(tc.tile_pool(name="psum_t", bufs=2, space="PSUM"))
        dram = ctx.enter_context(tc.tile_pool(name="dram", bufs=2, space="DRAM"))

        # ---- W slab, fp8 DoubleRow layout, 4 load pieces ----
        wdr8 = singles.tile([128, QCH, 2, R], FP8)
        WP = QCH // 4
        for q in range(4):
            qs = slice(q * WP, (q + 1) * WP)
            entry.append(nc.sync.dma_start(out=wdr8[:, qs], in_=w_dr[:, qs]))

        # ---- replicated per-node stats for ALL nodes, (p, c) layout ----
        # gst_full[p, c, b, s] (bf16): s=0 -> g, 1 -> g*ux, 2 -> g*uy, 3 -> 0
        fxf = small.tile([128, NCH, B], F32, tag="fxf", bufs=1)
        fyf = small.tile([128, NCH, B], F32, tag="fyf", bufs=1)
        dpf_r = dp_full.rearrange("t (p c) b -> t p c b", p=128)
        entry.append(nc.sync.dma_start(out=fxf, in_=dpf_r[0]))
        entry.append(nc.sync.dma_start(out=fyf, in_=dpf_r[1]))

        sqf = small.tile([128, NCH, B], F32, tag="sqf", bufs=1)
        tmf = small.tile([128, NCH, B], F32, tag="tmf", bufs=1)
        gff = small.tile([128, NCH, B], F32, tag="gff", bufs=1)
        nc.vector.tensor_mul(sqf, fxf, fxf)
        nc.vector.tensor_mul(tmf, fyf, fyf)
        nc.vector.tensor_add(sqf, sqf, tmf)
        gst_full = singles.tile([128, NCH, B, S4], BF16)
        nc.vector.memset(gst_full[:, :, :, 3], 0.0)
        nc.scalar.activation(gff, sqf, mybir.ActivationFunctionType.Exp,
                             scale=-0.5)
        nc.vector.tensor_copy(gst_full[:, :, :, 0], gff)
        nc.scalar.sqrt(tmf, sqf)
        rinf = small.tile([128, NCH, B], F32, tag="rinf", bufs=1)
        nc.vector.reciprocal(rinf, tmf)
        nc.vector.tensor_mul(gff, gff, rinf)
        nc.vector.tensor_mul(tmf, gff, fxf)
        nc.vector.tensor_copy(gst_full[:, :, :, 1], tmf)
        nc.vector.tensor_mul(tmf, gff, fyf)
        nc.vector.tensor_copy(gst_full[:, :, :, 2], tmf)

        # ---- combine coefficients for own rows, [p, d, b(bcast), s] ----
        # gcm2[p, d, s] = 0.5/WSCALE * [g, -g*ux, -g*uy, 0](i), i = d*128+p
        fx = small.tile([128, DCH, B], F32, tag="fx", bufs=1)
        fy = small.tile([128, DCH, B], F32, tag="fy", bufs=1)
        dp_r = dp_own.rearrange("t (d p) b -> t p d b", p=128)
        entry.append(nc.sync.dma_start(out=fx, in_=dp_r[0]))
        entry.append(nc.sync.dma_start(out=fy, in_=dp_r[1]))

        sq = small.tile([128, DCH, B], F32, tag="sq", bufs=1)
        t0 = small.tile([128, DCH, B], F32, tag="t0", bufs=1)
        nc.vector.tensor_mul(sq, fx, fx)
        nc.vector.tensor_mul(t0, fy, fy)
        nc.vector.tensor_add(sq, sq, t0)
        gcm2 = singles.tile([128, DCH, B, S4], F32)
        g_ = small.tile([128, DCH, B], F32, tag="g", bufs=1)
        nc.scalar.activation(g_, sq, mybir.ActivationFunctionType.Exp,
                             scale=-0.5)
        nc.scalar.sqrt(t0, sq)
        rin = small.tile([128, DCH, B], F32, tag="rin", bufs=1)
        nc.vector.reciprocal(rin, t0)
        nc.scalar.mul(gcm2[:, :, :, 0], g_, 0.5 / WSCALE)
        nc.vector.tensor_mul(g_, g_, rin)
        nc.vector.tensor_mul(t0, g_, fx)
        nc.scalar.mul(gcm2[:, :, :, 1], t0, -0.5 / WSCALE)
        nc.vector.tensor_mul(t0, g_, fy)
        nc.scalar.mul(gcm2[:, :, :, 2], t0, -0.5 / WSCALE)
        nc.vector.memset(gcm2[:, :, :, 3], 0.0)

        # ---- unary (own) + full logits (replicated) ----
        unary = singles.tile([128, DCH, B], F32)
        entry.append(nc.sync.dma_start(
            out=unary, in_=lg_own.rearrange("(d p) b -> p d b", p=128)))
        lgf = small.tile([128, NCH, B], F32, tag="lgf", bufs=1)
        entry.append(nc.sync.dma_start(
            out=lgf, in_=lg_full.rearrange("(p c) b -> p c b", p=128)))

        if chain_after is not None:
            for e in entry:
                add_dep_helper(e.ins, chain_after.ins,
                               reason="bench serialization")

        ident = singles.tile([128, 128], F32)
        make_identity(nc, ident)

        # X: [p, c, m] fp8, m = 4*b + s (s=3 lanes are the zero pad)
        xall_bufs = [singles.tile([128, NCH, MC], FP8, tag=f"xall{i}",
                                  name=f"xall{i}")
                     for i in range(2)]

        def build_x(xall_t, v_t):
            # xall[p, c, 4b+s] = gst_full[p, c, b, s] * v[p, c, b]
            nc.vector.tensor_mul(
                xall_t.rearrange("p c (b s) -> p c b s", s=S4),
                gst_full,
                v_t.broadcast_to([128, NCH, B, S4]))

        # ---- iteration 1 inputs computed locally (no collective) ----
        v0f = vpool.tile([128, NCH, B], BF16, tag="vf")
        nc.scalar.activation(v0f, lgf, mybir.ActivationFunctionType.Tanh,
                             scale=0.5)
        build_x(xall_bufs[0], v0f)

        lgt = None
        for it in range(ITER):
            xall = xall_bufs[it % 2]
            # y[m, i] = sum_j X[j, m] * wslab[j, i], j = p*32 + 2q + ko;
            # DoubleRow: lhsT [Ki=128, Ko=2, MC], rhs [Ki=128, Ko=2, R]
            y_ps = psum.tile([MC, 512], F32, tag="yps")
            for q in range(QCH):
                nc.tensor.matmul(y_ps,
                                 lhsT=xall[:, 2 * q:2 * q + 2, :],
                                 rhs=wdr8[:, q],
                                 start=(q == 0), stop=(q == QCH - 1),
                                 perf_mode=mybir.MatmulPerfMode.DoubleRow)

            # transpose to [p, (d, m)] via PE; all 4 blocks in one PSUM bank.
            # PSUM->SBUF copy split across DVE and ACT so the first
            # transposes start earlier and the engines work in parallel.
            y_sb = small.tile([MC, 512], F32, tag="ysb")
            nc.vector.tensor_copy(y_sb[:, :256], y_ps[:, :256])
            nc.scalar.activation(y_sb[:, 256:], y_ps[:, 256:],
                                 mybir.ActivationFunctionType.Copy)
            tp_all = psum_t.tile([128, DCH, MC], F32, tag="tp")
            for d in range(DCH):
                nc.tensor.transpose(tp_all[:, d],
                                    y_sb[:, d * 128:(d + 1) * 128],
                                    ident[:MC, :MC])

            # E = sum_s gcm2[p,d,s] * y^T[p,d,b,s];  logits = unary + E
            prod = small.tile([128, DCH, B, S4], F32, tag="prod")
            nc.vector.tensor_mul(
                prod, tp_all.rearrange("p d (b s) -> p d b s", s=S4), gcm2)
            e_t = small.tile([128, DCH, B], F32, tag="e")
            nc.vector.tensor_reduce(e_t, prod, mybir.AxisListType.X,
                                    mybir.AluOpType.add)
            lgt = small.tile([128, DCH, B], F32, tag="lgt")
            nc.vector.tensor_add(lgt, unary, e_t)

            if it < ITER - 1:
                # own v -> AllGather -> replicated full v -> next X
                v_own = small.tile([128, DCH, B], BF16, tag="vown")
                nc.scalar.activation(v_own, lgt,
                                     mybir.ActivationFunctionType.Tanh,
                                     scale=0.5)
                bounce_in = dram.tile([R, B], BF16, tag="bin")
                dma_out = nc.sync.dma_start(
                    out=bounce_in.rearrange("(d p) b -> p d b", p=128),
                    in_=v_own)
                bounce_out = dram.tile([N, B], BF16, tag="bout")
                if comm:
                    nc.gpsimd.collective_compute(
                        "AllGather",
                        mybir.AluOpType.bypass,
                        replica_groups=[list(range(NCORES))],
                        ins=[bounce_in.opt()],
                        outs=[bounce_out.opt()],
                    )
                else:
                    # single-core timing proxy: local copy instead of AllGather
                    nc.sync.dma_start(out=bounce_out[0:R, :], in_=bounce_in)
                # gather-back + X rebuild in halves so the DVE overlaps the
                # second DMA half
                vf = vpool.tile([128, NCH, B], BF16, tag="vf")
                bo_r = bounce_out.rearrange("(p c) b -> p c b", p=128)
                H = NCH // 2
                xn = xall_bufs[(it + 1) % 2]
                nc.sync.dma_start(out=vf[:, :H], in_=bo_r[:, :H])
                nc.vector.tensor_mul(
                    xn[:, :H].rearrange("p c (b s) -> p c b s", s=S4),
                    gst_full[:, :H],
                    vf[:, :H].broadcast_to([128, H, B, S4]))
                nc.sync.dma_start(out=vf[:, H:], in_=bo_r[:, H:])
                nc.vector.tensor_mul(
                    xn[:, H:].rearrange("p c (b s) -> p c b s", s=S4),
                    gst_full[:, H:],
                    vf[:, H:].broadcast_to([128, H, B, S4]))

                # pace the PE through the AllGather gap: group A fires right
                # after the transposes (covers ~[0, 1.7us] of the gap),
                # group B is gated on a DRAM->DRAM dummy DMA chained after
                # the bounce DMA, so it starts ~2.8us in and covers to
                # ~5us; HAM never sees a >3.4us idle window, and both
                # groups drain before the AllGather lands so the real
                # matmuls are never delayed.
                if WARM_B:
                    dummy_a = dram.tile([64], F32, tag="dummy_a")
                    dummy_b = dram.tile([64], F32, tag="dummy_b")
                    timer = nc.sync.dma_start(out=dummy_b, in_=dummy_a)
                    add_dep_helper(timer.ins, dma_out.ins,
                                   reason="warm-MM pacing timer")
                if WARM_A + WARM_B:
                    warm_ps = psum.tile([MC, 512], F32, tag="warm")
                for wi in range(WARM_A + WARM_B):
                    q = wi % QCH
                    mm = nc.tensor.matmul(warm_ps,
                                          lhsT=xall[:, 2 * q:2 * q + 2, :],
                                          rhs=wdr8[:, q],
                                          start=True, stop=True,
                                          perf_mode=mybir.MatmulPerfMode.DoubleRow)
                    if wi == WARM_A and WARM_B:
                        add_dep_helper(mm.ins, timer.ins,
                                       reason="pace warm MMs into the AG gap")

        return nc.sync.dma_start(
            out=out_own.rearrange("(d p) b -> p d b", p=128), in_=lgt)


_NC_CACHE = None


def _get_nc():
    global _NC_CACHE
    if _NC_CACHE is None:
        _NC_CACHE = _build_kernel()
    return _NC_CACHE


def kernel(delta_p, logits, W, _trace=False):
    in_maps = _make_in_maps(delta_p, logits, W)
    res = run_bass_kernel_spmd(_get_nc(), in_maps, core_ids=list(range(NCORES)),
                               trace=_trace)
    if _trace:
        kernel._last_result = res
    return _assemble_out(res)
